# revision 4
# baseline (speedup 1.0000x reference)
"""Multi-head attention (B=4, T=2048, D=768, H=12) on 8 trn2 cores.

Sharding: core c handles batch b=c//2, head-group g=c%2 (6 heads each).
Host does layout prep (transpose/reshape of shards) and the final 2-way
partial-sum per batch (the "all-reduce" of the row-split output proj).

Per-core pipeline (all matmuls in float32r):
  qT/kT  [128=(2 heads x 64d), T] feature-major   (PE, K=768 in 6 tiles)
  v_aug  [T, 65] t-major with ones column         (PE)
  scoresT[j,i] = kT.qT  row-packed head pairs     (PE, K=64 x2 concurrent)
  E = exp(scores/8)                               (ACT, reads PSUM 1024-wide)
  accT[65, i] += v_aug.T @ E  (row 64 = rowsum)   (PE, PSUM accum over j)
  outT = accT[0:64]/accT[64] -> proj lhsT         (DVE recip/mul + gpsimd bcast)
  out[e,t] = wo.T @ outT + b                      (PE + DVE per-partition add)
"""
import numpy as np

B, T, C = 4, 2048, 768
H, D = 12, 64
HL, NP = 6, 3          # local heads, head pairs per core
KT = 6                 # k tiles over C
JT = 16                # j tiles over T
NIC, ICW = 4, 512      # i chunks
N_CORES = 8

_CACHE = {}


def _build(reps=None):
    import concourse.bacc as bacc
    import concourse.tile as tile
    import concourse.mybir as mybir

    f32 = mybir.dt.float32
    f32r = mybir.dt.float32r
    EXP = mybir.ActivationFunctionType.Exp
    ADD = mybir.AluOpType.add

    nc = bacc.Bacc("TRN2", target_bir_lowering=False, debug=False,
                   num_devices=N_CORES)

    xT_d = nc.dram_tensor("xT", [C, T], f32r, kind="ExternalInput")
    wq_d = nc.dram_tensor("wq", [128, 2304], f32r, kind="ExternalInput")
    wk_d = nc.dram_tensor("wk", [128, 2304], f32r, kind="ExternalInput")
    wv_d = nc.dram_tensor("wv", [128, 2304], f32r, kind="ExternalInput")
    wo_d = nc.dram_tensor("wo", [128, 2304], f32r, kind="ExternalInput")
    b_d = nc.dram_tensor("bo", [128, 6], f32, kind="ExternalInput")
    out_d = nc.dram_tensor("out", [C, T], f32, kind="ExternalOutput")

    with tile.TileContext(nc) as tc:
        from contextlib import ExitStack
        with ExitStack() as ctx:
            xpool = ctx.enter_context(tc.tile_pool(name="xp", bufs=KT))
            cpool = ctx.enter_context(tc.tile_pool(name="cp", bufs=1))
            qpool = ctx.enter_context(tc.tile_pool(name="qp", bufs=2))
            kpool = ctx.enter_context(tc.tile_pool(name="kp", bufs=2))
            epool = ctx.enter_context(tc.tile_pool(name="ep", bufs=3))
            ppool = ctx.enter_context(tc.tile_pool(name="pp", bufs=1))
            rcpool = ctx.enter_context(tc.tile_pool(name="rc", bufs=2))
            repool = ctx.enter_context(tc.tile_pool(name="re", bufs=2))
            opool = ctx.enter_context(tc.tile_pool(name="op", bufs=3))
            big = ctx.enter_context(tc.tile_pool(name="bg", bufs=3, space="PSUM"))
            accp = ctx.enter_context(tc.tile_pool(name="ac", bufs=2, space="PSUM"))

            def body():
                # --- input DMA ---
                xts = []
                for k in range(KT):
                    xt = xpool.tile([128, T], f32r, tag="x", name=f"x{k}")
                    nc.sync.dma_start(xt[:], xT_d.ap()[k * 128:(k + 1) * 128, :])
                    xts.append(xt)
                wq_sb = cpool.tile([128, 2304], f32r, tag="wq", name="wq_sb")
                wk_sb = cpool.tile([128, 2304], f32r, tag="wk", name="wk_sb")
                wv_sb = cpool.tile([128, 2304], f32r, tag="wv", name="wv_sb")
                wo_sb = cpool.tile([128, 2304], f32r, tag="wo", name="wo_sb")
                b_sb = cpool.tile([128, 6], f32, tag="bo", name="b_sb")
                for t_, d_ in ((wq_sb, wq_d), (wk_sb, wk_d), (wv_sb, wv_d),
                               (wo_sb, wo_d), (b_sb, b_d)):
                    nc.sync.dma_start(t_[:], d_.ap()[:, :])
                # v_aug: per j-tile x pair x head: 65 cols (64 v + ones)
                v_sb = cpool.tile([128, JT * 390], f32r, tag="vs", name="v_sb")
                ones_view = v_sb[:].rearrange("p (a x) -> p a x", x=65)[:, :, 64:65]
                nc.vector.memset(ones_view.bitcast(f32), 1.0)

                qts, kts = {}, {}

                def emit_qk(p):
                    for wsb, pool, store, nm in ((wq_sb, qpool, qts, "qT"),
                                                 (wk_sb, kpool, kts, "kT")):
                        dst = pool.tile([128, T], f32r, tag=nm, name=f"{nm}{p}")
                        store[p] = dst
                        for half in range(2):
                            ps = big.tile([128, 1024], f32, tag="big",
                                          name=f"ps_{nm}{p}_{half}")
                            for sub in range(2):
                                o = ps[:, sub * 512:(sub + 1) * 512]
                                c0 = half * 1024 + sub * 512
                                for k in range(KT):
                                    nc.tensor.matmul(
                                        o,
                                        lhsT=wsb[:, (k * 3 + p) * 128:(k * 3 + p + 1) * 128],
                                        rhs=xts[k][:, c0:c0 + 512],
                                        start=(k == 0), stop=(k == KT - 1))
                                yield
                            nc.vector.tensor_copy(
                                dst[:, half * 1024:(half + 1) * 1024], ps[:])
                            yield

                def emit_v():
                    for tt2 in range(8):
                        ps = big.tile([128, 1024], f32, tag="big", name=f"ps_v{tt2}")
                        for sub in range(2):
                            tt = tt2 * 2 + sub
                            o = ps[:, sub * 512:sub * 512 + 384]
                            for k in range(KT):
                                nc.tensor.matmul(
                                    o,
                                    lhsT=xts[k][:, tt * 128:(tt + 1) * 128],
                                    rhs=wv_sb[:, k * 384:(k + 1) * 384],
                                    start=(k == 0), stop=(k == KT - 1))
                        for sub in range(2):
                            tt = tt2 * 2 + sub
                            for p in range(NP):
                                src = ps[:, sub * 512 + p * 128:sub * 512 + (p + 1) * 128]
                                src = src.rearrange("a (h d) -> a h d", h=2)
                                dst = v_sb[:, tt * 390 + p * 130:tt * 390 + (p + 1) * 130]
                                dst = dst.rearrange("a (h x) -> a h x", h=2)[:, :, 0:64]
                                nc.vector.tensor_copy(dst, src)

                def emit_proj_chunk(tch):
                    for et in range(6):
                        ps = big.tile([128, 1024], f32, tag="big",
                                      name=f"ps_o{tch}_{et}")
                        o = ps[:, 0:512]
                        for p in range(NP):
                            nc.tensor.matmul(
                                o,
                                lhsT=wo_sb[:, p * 768 + et * 128:p * 768 + (et + 1) * 128],
                                rhs=proj_sb[p][:, tch * 512:(tch + 1) * 512],
                                start=(p == 0), stop=(p == NP - 1))
                        ot = opool.tile([128, 512], f32, tag="ost", name=f"o{tch}_{et}")
                        nc.vector.tensor_scalar(ot[:], o, b_sb[:, et:et + 1], None,
                                                op0=ADD)
                        nc.sync.dma_start(
                            out_d.ap()[et * 128:(et + 1) * 128,
                                       tch * 512:(tch + 1) * 512], ot[:])

                def emit_attn(p, interleave=None, do_proj=False):
                    qT, kT = qts[p], kts[p]
                    for ic in range(NIC):
                        acc = [accp.tile([65, 512], f32, tag="acc",
                                         name=f"acc{p}_{ic}_{hh}")
                               for hh in range(2)]
                        for jg in range(8):
                            ets = []
                            for hh in range(2):
                                sc = big.tile([128, 1024], f32, tag="big",
                                              name=f"sc{p}_{ic}_{jg}_{hh}")
                                for jt in range(2):
                                    j = jg * 2 + jt
                                    nc.tensor.matmul(
                                        sc[:, jt * 512:(jt + 1) * 512],
                                        lhsT=kT[hh * 64:(hh + 1) * 64,
                                                j * 128:(j + 1) * 128],
                                        rhs=qT[hh * 64:(hh + 1) * 64,
                                               ic * 512:(ic + 1) * 512],
                                        start=True, stop=True)
                                et = epool.tile([128, 1024], f32r, tag="et",
                                                name=f"et{p}_{ic}_{jg}_{hh}")
                                nc.scalar.activation(et[:], sc[:], EXP, scale=0.125)
                                ets.append(et)
                            for hh in range(2):
                                for jt in range(2):
                                    j = jg * 2 + jt
                                    v0 = j * 390 + p * 130 + hh * 65
                                    nc.tensor.matmul(
                                        acc[hh][:, :],
                                        lhsT=v_sb[:, v0:v0 + 65],
                                        rhs=ets[hh][:, jt * 512:(jt + 1) * 512],
                                        start=(jg == 0 and jt == 0),
                                        stop=(jg == 7 and jt == 1))
                            if interleave is not None:
                                next(interleave, None)
                        for hh in range(2):
                            rcp = rcpool.tile([1, 512], f32, tag="rcp",
                                              name=f"rcp{p}_{ic}_{hh}")
                            nc.vector.reciprocal(rcp[:], acc[hh][64:65, :])
                            rep = repool.tile([64, 512], f32, tag="rep",
                                              name=f"rep{p}_{ic}_{hh}")
                            nc.gpsimd.partition_broadcast(rep[:], rcp[:])
                            nc.vector.tensor_mul(
                                proj_sb[p][hh * 64:(hh + 1) * 64,
                                           ic * 512:(ic + 1) * 512],
                                acc[hh][0:64, :], rep[:])
                        if do_proj:
                            emit_proj_chunk(ic)

                proj_sb = [ppool.tile([128, T], f32r, tag=f"pj{p}", name=f"pj{p}")
                           for p in range(NP)]

                for _ in emit_qk(0):
                    pass
                emit_v()
                qk1 = emit_qk(1)
                emit_attn(0, interleave=qk1)
                for _ in qk1:
                    pass
                qk2 = emit_qk(2)
                emit_attn(1, interleave=qk2)
                for _ in qk2:
                    pass
                emit_attn(2, do_proj=True)

            if reps is None:
                body()
            else:
                with tc.For_i(0, reps, 1):
                    body()

    nc.compile()
    return nc


def _chain(*gens):
    for g in gens:
        yield from g


def _prep_core(x, w_qkv, w_out, b_out, b, g):
    """Host-side layout prep for core (batch b, head-group g)."""
    f = np.float32
    xT = np.ascontiguousarray(x[b].T, dtype=f)                  # (768, 2048)
    W4 = w_qkv.reshape(C, D, 3, H)                              # [c, d, k, h]
    hs = slice(g * HL, (g + 1) * HL)
    # wq/wk: [c128, (k, pair, hh, d)]
    def qk_layout(kk):
        A = W4[:, :, kk, hs]                                    # (768, 64, 6)
        A = A.reshape(KT, 128, D, NP, 2)                        # (k, c, d, p, hh)
        return np.ascontiguousarray(
            A.transpose(1, 0, 3, 4, 2).reshape(128, 2304), dtype=f)
    wq = qk_layout(0)
    wk = qk_layout(1)
    Av = W4[:, :, 2, hs].reshape(KT, 128, D, HL)                # (k, c, d, h)
    wv = np.ascontiguousarray(
        Av.transpose(1, 0, 3, 2).reshape(128, 2304), dtype=f)   # [c128,(k,h,d)]
    Ao = w_out.reshape(H, D, C)[hs]                             # (6, 64, 768)
    Ao = Ao.reshape(NP, 2, D, C)                                # (p, hh, d, e)
    wo = np.ascontiguousarray(
        Ao.transpose(1, 2, 0, 3).reshape(128, 2304), dtype=f)   # [(hh,d),(p,e)]
    if g == 0:
        bo = np.ascontiguousarray(b_out.reshape(6, 128).T, dtype=f)
    else:
        bo = np.zeros((128, 6), dtype=f)
    return {"xT": xT, "wq": wq, "wk": wk, "wv": wv, "wo": wo, "bo": bo}


def make_in_maps(x, w_qkv, w_out, b_out):
    return [_prep_core(x, w_qkv, w_out, b_out, c // 2, c % 2)
            for c in range(N_CORES)]


def kernel(x, w_qkv, w_out, b_out):
    from concourse import bass_utils
    if "nc" not in _CACHE:
        _CACHE["nc"] = _build()
    nc = _CACHE["nc"]
    in_maps = make_in_maps(np.asarray(x, dtype=np.float32),
                           np.asarray(w_qkv, dtype=np.float32),
                           np.asarray(w_out, dtype=np.float32),
                           np.asarray(b_out, dtype=np.float32))
    res = bass_utils.run_bass_kernel_spmd(nc, in_maps,
                                          core_ids=list(range(N_CORES)))
    out = np.empty((B, T, C), dtype=np.float32)
    for b in range(B):
        s = res.results[2 * b]["out"] + res.results[2 * b + 1]["out"]
        out[b] = s.T
    return out
